# revision 5
# baseline (speedup 1.0000x reference)
"""Trainium2 Bass kernel for y = x @ W.T + b  (x: [16384,1024] f32,
W: [1024,1024] f32, b: [1024] f32) on 8 NeuronCores.

Data-parallel: x is split along batch into 8 shards of 2048 rows;
W and b are replicated. Each core computes its y shard with bf16
matmuls accumulating in fp32 PSUM; bias is fused into the PSUM->SBUF
eviction on the Scalar engine. Host-side we pre-transpose x (and W) to
put the contraction dim on SBUF partitions, so no on-chip transposes
are needed.

Loop order per core: batch-chunk (bq, 4 x 512 cols) outer, output-tile
(mo, 8 x 128 rows) middle, contraction (ko, 8 x 128) inner. Each
128 KiB x chunk is reused for all 8 mo tiles as soon as it lands, so
compute starts ~1.5 us after the first chunks instead of racing the
full 6 MiB input load; each (bq, mo) PSUM bank is evicted and DMA'd
out while later tiles compute, so there is no output pile-up at the
tail.
"""

import sys

if "/opt/trn_rl_repo" not in sys.path:
    sys.path.insert(0, "/opt/trn_rl_repo")

import ml_dtypes
import numpy as np

BATCH = 16384
IN_F = 1024
OUT_F = 1024
NCORES = 8
P = 128
KO = IN_F // P  # 8 contraction tiles
MO = OUT_F // P  # 8 output-feature tiles
BS = BATCH // NCORES  # 2048 rows per core
FD = 512  # matmul moving free dim (one PSUM bank of fp32)
NB = BS // FD  # 4 moving chunks per core

_cache = {}
LAST_RESULT = None


def _build():
    import concourse.mybir as mybir
    import concourse.tile as tile
    from concourse import bacc

    nc = bacc.Bacc(None, target_bir_lowering=False)
    xT = nc.declare_dram_parameter("xT", [P, KO, BS], mybir.dt.bfloat16, isOutput=False)
    # w2[p, mo, ko, c] = W[mo*P + c, ko*P + p] — per-mo contiguous so the
    # first mo sweep only gates on a 256 KiB chunk.
    w2 = nc.declare_dram_parameter(
        "w2", [P, MO, KO, P], mybir.dt.bfloat16, isOutput=False
    )
    bias = nc.declare_dram_parameter("bias", [P, MO], mybir.dt.float32, isOutput=False)
    out = nc.declare_dram_parameter("out", [P, MO, BS], mybir.dt.float32, isOutput=True)

    with tile.TileContext(nc) as tc:
        with (
            tc.tile_pool(name="const", bufs=1) as cpool,
            tc.tile_pool(name="outp", bufs=4) as opool,
            tc.tile_pool(name="psum", bufs=6, space="PSUM") as ppool,
        ):
            x_sb = cpool.tile([P, KO, BS], mybir.dt.bfloat16)
            w_sb = cpool.tile([P, MO, KO, P], mybir.dt.bfloat16)
            b_sb = cpool.tile([P, MO], mybir.dt.float32)
            # Each dma_start costs ~0.65 us of sequencer issue time and the
            # first ~3 MiB of input is needed within ~10 us, so the issue
            # work is split across two queue engines: weights per-mo on
            # GpSimd, x chunks on Sync, both in consumption order.
            for mo in range(MO):
                nc.gpsimd.dma_start(w_sb[:, mo], w2[:, mo])
            nc.gpsimd.dma_start(b_sb[:], bias[:])
            for ko in range(KO):
                nc.sync.dma_start(x_sb[:, ko, 0:FD], xT[:, ko, 0:FD])
            for bq in range(1, NB):
                nc.sync.dma_start(
                    x_sb[:, :, bq * FD : (bq + 1) * FD],
                    xT[:, :, bq * FD : (bq + 1) * FD],
                )

            for bq in range(NB):
                bsl = slice(bq * FD, (bq + 1) * FD)
                o_sb = opool.tile([P, MO, FD], mybir.dt.float32)
                for mo in range(MO):
                    ps = ppool.tile([P, FD], mybir.dt.float32)
                    for ko in range(KO):
                        nc.tensor.matmul(
                            ps[:],
                            w_sb[:, mo, ko],
                            x_sb[:, ko, bsl],
                            start=(ko == 0),
                            stop=(ko == KO - 1),
                        )
                    nc.scalar.activation(
                        o_sb[:, mo],
                        ps[:],
                        mybir.ActivationFunctionType.Identity,
                        bias=b_sb[:, mo : mo + 1],
                    )
                if bq < NB - 1:
                    nc.sync.dma_start(out[:, :, bsl], o_sb[:])
                else:
                    # Finer pushes on the last chunk so the final store
                    # doesn't add a 1 MiB DMA to the kernel tail.
                    for mh in range(0, MO, 2):
                        nc.sync.dma_start(
                            out[:, mh : mh + 2, bsl], o_sb[:, mh : mh + 2]
                        )

    nc.compile()
    return nc


def kernel(x, weight, bias):
    global LAST_RESULT
    from concourse.bass_utils import run_bass_kernel_spmd

    if "nc" not in _cache:
        _cache["nc"] = _build()
    nc = _cache["nc"]

    bf16 = ml_dtypes.bfloat16
    # w2[p, mo, ko, c] = W[mo*P + c, ko*P + p]
    wb = weight.astype(bf16).reshape(MO, P, KO, P)  # [mo, c, ko, p]
    w2 = np.ascontiguousarray(wb.transpose(3, 0, 2, 1))  # [p, mo, ko, c]
    # bias laid out [P, MO]: b[p, mo] = bias[mo*P + p]
    b_t = np.ascontiguousarray(bias.astype(np.float32).reshape(MO, P).T)

    in_maps = []
    for c in range(NCORES):
        xs = x[c * BS : (c + 1) * BS].astype(bf16)
        # x.T laid out [P, KO, BS]: xT[p, ko, b] = x[b, ko*P + p]
        xT = np.ascontiguousarray(xs.T.reshape(KO, P, BS).transpose(1, 0, 2))
        in_maps.append({"xT": xT, "w2": w2, "bias": b_t})

    res = run_bass_kernel_spmd(nc, in_maps, list(range(NCORES)))
    LAST_RESULT = res

    y = np.empty((BATCH, OUT_F), dtype=np.float32)
    for c in range(NCORES):
        o = res.results[c]["out"]  # [P, MO, BS]
        y[c * BS : (c + 1) * BS] = o.transpose(2, 1, 0).reshape(BS, OUT_F)
    return y


# revision 6
# speedup vs baseline: 1.1245x; 1.1245x over previous
"""Trainium2 Bass kernel for y = x @ W.T + b  (x: [16384,1024] f32,
W: [1024,1024] f32, b: [1024] f32) on 8 NeuronCores.

Data-parallel: x is split along batch into 8 shards of 2048 rows;
W and b are replicated. Each core computes its y shard with bf16
matmuls accumulating in fp32 PSUM; bias is fused into the PSUM->SBUF
eviction on the Scalar engine. Host-side we pre-transpose x (and W) to
put the contraction dim on SBUF partitions, so no on-chip transposes
are needed, and group DRAM layouts by batch-chunk so every DMA is 128
long contiguous runs (DMA issue time scales with descriptor rows).

Loop order per core: batch-chunk (bq, 4 x 512 cols) outer, output-tile
(mo, 8 x 128 rows) middle, contraction (ko, 8 x 128) inner. Each x
chunk is reused for all 8 mo tiles as soon as it lands, so compute
starts ~10 us in (right after the NEFF init barrier + first chunks)
and never starves; each (bq, mo) PSUM bank is evicted while later
tiles compute, and outputs stream out per batch-chunk with the last
chunk split fine so the kernel tail stays short.
"""

import sys

if "/opt/trn_rl_repo" not in sys.path:
    sys.path.insert(0, "/opt/trn_rl_repo")

import ml_dtypes
import numpy as np

BATCH = 16384
IN_F = 1024
OUT_F = 1024
NCORES = 8
P = 128
KO = IN_F // P  # 8 contraction tiles
MO = OUT_F // P  # 8 output-feature tiles
BS = BATCH // NCORES  # 2048 rows per core
FD = 512  # matmul moving free dim (one PSUM bank of fp32)
NB = BS // FD  # 4 batch chunks per core

_cache = {}
LAST_RESULT = None


def _build():
    import concourse.mybir as mybir
    import concourse.tile as tile
    from concourse import bacc

    nc = bacc.Bacc(None, target_bir_lowering=False)
    # xT4[p, bq, ko, fd] = x[bq*FD + fd, ko*P + p]
    xT = nc.declare_dram_parameter(
        "xT", [P, NB, KO, FD], mybir.dt.bfloat16, isOutput=False
    )
    # w2[p, mo, ko, c] = W[mo*P + c, ko*P + p]
    w2 = nc.declare_dram_parameter(
        "w2", [P, MO, KO, P], mybir.dt.bfloat16, isOutput=False
    )
    bias = nc.declare_dram_parameter("bias", [P, MO], mybir.dt.float32, isOutput=False)
    # out4[p, bq, mo, fd] = y[bq*FD + fd, mo*P + p]
    out = nc.declare_dram_parameter(
        "out", [P, NB, MO, FD], mybir.dt.float32, isOutput=True
    )

    with tile.TileContext(nc) as tc:
        with (
            tc.tile_pool(name="const", bufs=1) as cpool,
            tc.tile_pool(name="outp", bufs=3) as opool,
            tc.tile_pool(name="psum", bufs=6, space="PSUM") as ppool,
        ):
            x_sb = cpool.tile([P, NB, KO, FD], mybir.dt.bfloat16)
            w_sb = cpool.tile([P, MO, KO, P], mybir.dt.bfloat16)
            b_sb = cpool.tile([P, MO], mybir.dt.float32)
            # Issue order interleaves weights and bq0 x chunks (the inputs
            # the first output tiles gate on), then the remaining batch
            # chunks as one large contiguous DMA each.
            nc.sync.dma_start(w_sb[:, 0], w2[:, 0])
            nc.sync.dma_start(x_sb[:, 0, 0:2], xT[:, 0, 0:2])
            nc.sync.dma_start(w_sb[:, 1], w2[:, 1])
            nc.sync.dma_start(x_sb[:, 0, 2:4], xT[:, 0, 2:4])
            nc.sync.dma_start(w_sb[:, 2], w2[:, 2])
            nc.sync.dma_start(x_sb[:, 0, 4:6], xT[:, 0, 4:6])
            nc.sync.dma_start(w_sb[:, 3], w2[:, 3])
            nc.sync.dma_start(x_sb[:, 0, 6:8], xT[:, 0, 6:8])
            for mo in range(4, MO):
                nc.sync.dma_start(w_sb[:, mo], w2[:, mo])
            nc.sync.dma_start(b_sb[:], bias[:])
            for bq in range(1, NB):
                nc.sync.dma_start(x_sb[:, bq], xT[:, bq])

            for bq in range(NB):
                o_sb = opool.tile([P, MO, FD], mybir.dt.float32)
                for mo in range(MO):
                    ps = ppool.tile([P, FD], mybir.dt.float32)
                    for ko in range(KO):
                        nc.tensor.matmul(
                            ps[:],
                            w_sb[:, mo, ko],
                            x_sb[:, bq, ko],
                            start=(ko == 0),
                            stop=(ko == KO - 1),
                        )
                    nc.scalar.activation(
                        o_sb[:, mo],
                        ps[:],
                        mybir.ActivationFunctionType.Identity,
                        bias=b_sb[:, mo : mo + 1],
                    )
                if bq < NB - 1:
                    nc.sync.dma_start(out[:, bq], o_sb[:])
                else:
                    # Finer pushes on the last chunk so the final store
                    # doesn't add a 1 MiB DMA to the kernel tail.
                    for mh in range(0, MO, 2):
                        nc.sync.dma_start(
                            out[:, bq, mh : mh + 2], o_sb[:, mh : mh + 2]
                        )

    nc.compile()
    return nc


def kernel(x, weight, bias):
    global LAST_RESULT
    from concourse.bass_utils import run_bass_kernel_spmd

    if "nc" not in _cache:
        _cache["nc"] = _build()
    nc = _cache["nc"]

    bf16 = ml_dtypes.bfloat16
    # w2[p, mo, ko, c] = W[mo*P + c, ko*P + p]
    wb = weight.astype(bf16).reshape(MO, P, KO, P)  # [mo, c, ko, p]
    w2 = np.ascontiguousarray(wb.transpose(3, 0, 2, 1))  # [p, mo, ko, c]
    # bias laid out [P, MO]: b[p, mo] = bias[mo*P + p]
    b_t = np.ascontiguousarray(bias.astype(np.float32).reshape(MO, P).T)

    in_maps = []
    for c in range(NCORES):
        xs = x[c * BS : (c + 1) * BS].astype(bf16)
        # xT4[p, bq, ko, fd] = x[bq*FD + fd, ko*P + p]
        xr = xs.reshape(NB, FD, KO, P)  # [bq, fd, ko, p]
        xT = np.ascontiguousarray(xr.transpose(3, 0, 2, 1))  # [p, bq, ko, fd]
        in_maps.append({"xT": xT, "w2": w2, "bias": b_t})

    res = run_bass_kernel_spmd(nc, in_maps, list(range(NCORES)))
    LAST_RESULT = res

    y = np.empty((BATCH, OUT_F), dtype=np.float32)
    for c in range(NCORES):
        o = res.results[c]["out"]  # [p, bq, mo, fd]
        y[c * BS : (c + 1) * BS] = o.transpose(1, 3, 2, 0).reshape(BS, OUT_F)
    return y


# revision 9
# speedup vs baseline: 1.1329x; 1.0075x over previous
"""Trainium2 Bass kernel for y = x @ W.T + b  (x: [16384,1024] f32,
W: [1024,1024] f32, b: [1024] f32) on 8 NeuronCores.

Data-parallel: x is split along batch into 8 shards of 2048 rows;
W and b are replicated. Each core computes its y shard with bf16
matmuls accumulating in fp32 PSUM; bias is fused into the PSUM->SBUF
eviction on the Scalar engine. Host-side we pre-transpose x (and W) to
put the contraction dim on SBUF partitions, so no on-chip transposes
are needed, and group DRAM layouts by batch-chunk so every DMA is 128
long contiguous runs (DMA issue time scales with descriptor rows).

Loop order per core: batch-chunk (bq, 4 x 512 cols) outer, output-tile
(mo, 8 x 128 rows) middle, contraction (ko, 8 x 128) inner. Each x
chunk is reused for all 8 mo tiles as soon as it lands, so compute
starts ~10 us in (right after the NEFF init barrier + first chunks)
and never starves; each (bq, mo) PSUM bank is evicted while later
tiles compute, and outputs stream out per batch-chunk with the last
chunk split fine so the kernel tail stays short.
"""

import sys

if "/opt/trn_rl_repo" not in sys.path:
    sys.path.insert(0, "/opt/trn_rl_repo")

import ml_dtypes
import numpy as np

BATCH = 16384
IN_F = 1024
OUT_F = 1024
NCORES = 8
P = 128
KO = IN_F // P  # 8 contraction tiles
MO = OUT_F // P  # 8 output-feature tiles
BS = BATCH // NCORES  # 2048 rows per core
FD = 512  # matmul moving free dim (one PSUM bank of fp32)
NB = BS // FD  # 4 batch chunks per core

_cache = {}
LAST_RESULT = None


def _build():
    import concourse.mybir as mybir
    import concourse.tile as tile
    from concourse import bacc

    nc = bacc.Bacc(None, target_bir_lowering=False)
    # xT4[p, bq, ko, fd] = x[bq*FD + fd, ko*P + p]
    xT = nc.declare_dram_parameter(
        "xT", [P, NB, KO, FD], mybir.dt.bfloat16, isOutput=False
    )
    # w2[p, mo, ko, c] = W[mo*P + c, ko*P + p]
    w2 = nc.declare_dram_parameter(
        "w2", [P, MO, KO, P], mybir.dt.bfloat16, isOutput=False
    )
    bias = nc.declare_dram_parameter("bias", [P, MO], mybir.dt.float32, isOutput=False)
    # out4[p, bq, mo, fd] = y[bq*FD + fd, mo*P + p]
    out = nc.declare_dram_parameter(
        "out", [P, NB, MO, FD], mybir.dt.float32, isOutput=True
    )

    with tile.TileContext(nc) as tc:
        with (
            tc.tile_pool(name="const", bufs=1) as cpool,
            tc.tile_pool(name="outp", bufs=3) as opool,
            tc.tile_pool(name="psum", bufs=6, space="PSUM") as ppool,
            tc.tile_pool(name="warmps", bufs=1, space="PSUM") as wpool,
        ):
            x_sb = cpool.tile([P, NB, KO, FD], mybir.dt.bfloat16)
            w_sb = cpool.tile([P, MO, KO, P], mybir.dt.bfloat16)
            b_sb = cpool.tile([P, MO], mybir.dt.float32)
            # PE HAM warm-up: the PE clock is gated to 1.2 GHz until it has
            # been busy ~3.4 us. Burn that window on dummy matmuls over a
            # zeroed tile while the first input DMAs are still in flight,
            # so the real matmul stream runs at 2.4 GHz from its first
            # instruction.
            wu_sb = cpool.tile([P, 256], mybir.dt.bfloat16)
            nc.any.memset(wu_sb[:], 0.0)
            wu_ps = wpool.tile([P, 256], mybir.dt.float32)
            for _ in range(16):
                nc.tensor.matmul(
                    wu_ps[:], wu_sb[:, :P], wu_sb[:], start=True, stop=True
                )
            # Issue order interleaves weights and bq0 x chunks (the inputs
            # the first output tiles gate on), then the remaining batch
            # chunks as one large contiguous DMA each.
            nc.sync.dma_start(w_sb[:, 0], w2[:, 0])
            nc.sync.dma_start(x_sb[:, 0, 0:2], xT[:, 0, 0:2])
            nc.sync.dma_start(w_sb[:, 1], w2[:, 1])
            nc.sync.dma_start(x_sb[:, 0, 2:4], xT[:, 0, 2:4])
            nc.sync.dma_start(w_sb[:, 2], w2[:, 2])
            nc.sync.dma_start(x_sb[:, 0, 4:6], xT[:, 0, 4:6])
            nc.sync.dma_start(w_sb[:, 3], w2[:, 3])
            nc.sync.dma_start(x_sb[:, 0, 6:8], xT[:, 0, 6:8])
            for mo in range(4, MO):
                nc.sync.dma_start(w_sb[:, mo], w2[:, mo])
            nc.sync.dma_start(b_sb[:], bias[:])
            for bq in range(1, NB):
                nc.sync.dma_start(x_sb[:, bq], xT[:, bq])

            for bq in range(NB):
                o_sb = opool.tile([P, MO, FD], mybir.dt.float32)
                for mo in range(MO):
                    ps = ppool.tile([P, FD], mybir.dt.float32)
                    for ko in range(KO):
                        nc.tensor.matmul(
                            ps[:],
                            w_sb[:, mo, ko],
                            x_sb[:, bq, ko],
                            start=(ko == 0),
                            stop=(ko == KO - 1),
                        )
                    nc.scalar.activation(
                        o_sb[:, mo],
                        ps[:],
                        mybir.ActivationFunctionType.Identity,
                        bias=b_sb[:, mo : mo + 1],
                    )
                if bq < NB - 1:
                    nc.sync.dma_start(out[:, bq], o_sb[:])
                else:
                    # Finer pushes on the last chunk so the final store
                    # doesn't add a 1 MiB DMA to the kernel tail.
                    for mh in range(0, MO, 2):
                        nc.sync.dma_start(
                            out[:, bq, mh : mh + 2], o_sb[:, mh : mh + 2]
                        )

    nc.compile()
    return nc


def kernel(x, weight, bias):
    global LAST_RESULT
    from concourse.bass_utils import run_bass_kernel_spmd

    if "nc" not in _cache:
        _cache["nc"] = _build()
    nc = _cache["nc"]

    bf16 = ml_dtypes.bfloat16
    # w2[p, mo, ko, c] = W[mo*P + c, ko*P + p]
    wb = weight.astype(bf16).reshape(MO, P, KO, P)  # [mo, c, ko, p]
    w2 = np.ascontiguousarray(wb.transpose(3, 0, 2, 1))  # [p, mo, ko, c]
    # bias laid out [P, MO]: b[p, mo] = bias[mo*P + p]
    b_t = np.ascontiguousarray(bias.astype(np.float32).reshape(MO, P).T)

    in_maps = []
    for c in range(NCORES):
        xs = x[c * BS : (c + 1) * BS].astype(bf16)
        # xT4[p, bq, ko, fd] = x[bq*FD + fd, ko*P + p]
        xr = xs.reshape(NB, FD, KO, P)  # [bq, fd, ko, p]
        xT = np.ascontiguousarray(xr.transpose(3, 0, 2, 1))  # [p, bq, ko, fd]
        in_maps.append({"xT": xT, "w2": w2, "bias": b_t})

    res = run_bass_kernel_spmd(nc, in_maps, list(range(NCORES)))
    LAST_RESULT = res

    y = np.empty((BATCH, OUT_F), dtype=np.float32)
    for c in range(NCORES):
        o = res.results[c]["out"]  # [p, bq, mo, fd]
        y[c * BS : (c + 1) * BS] = o.transpose(1, 3, 2, 0).reshape(BS, OUT_F)
    return y


# revision 11
# speedup vs baseline: 1.1479x; 1.0132x over previous
"""Trainium2 Bass kernel for y = x @ W.T + b  (x: [16384,1024] f32,
W: [1024,1024] f32, b: [1024] f32) on 8 NeuronCores.

Data-parallel: x is split along batch into 8 shards of 2048 rows;
W and b are replicated. Each core computes its y shard with bf16
matmuls accumulating in fp32 PSUM; bias is fused into the PSUM->SBUF
eviction on the Scalar engine. Host-side we pre-transpose x (and W) to
put the contraction dim on SBUF partitions, so no on-chip transposes
are needed, and group DRAM layouts so every DMA is 128 long contiguous
runs (DMA issue time scales with descriptor rows).

Schedule per core (bq = one of 4 batch chunks of 512 rows):
- Dummy matmuls warm the PE clock gate (1.2 -> 2.4 GHz takes ~3.4 us
  of busy) while the first input DMAs are in flight.
- bq0 runs contraction-outer across all 8 PSUM banks, consuming one
  (w[ko], x[ko]) chunk pair per 1.7 us — matched to the observed DMA
  rate so the matmul stream never starves while inputs land.
- bq1..3 run output-tile-outer (one PSUM bank at a time), evicting
  each bank through the Scalar engine while later tiles compute.
- Outputs stream out per batch chunk; the last chunk is split into
  four stores so the final DMA doesn't lengthen the kernel tail.
"""

import sys

if "/opt/trn_rl_repo" not in sys.path:
    sys.path.insert(0, "/opt/trn_rl_repo")

import ml_dtypes
import numpy as np

BATCH = 16384
IN_F = 1024
OUT_F = 1024
NCORES = 8
P = 128
KO = IN_F // P  # 8 contraction tiles
MO = OUT_F // P  # 8 output-feature tiles
BS = BATCH // NCORES  # 2048 rows per core
FD = 512  # matmul moving free dim (one PSUM bank of fp32)
NB = BS // FD  # 4 batch chunks per core

_cache = {}
LAST_RESULT = None


def _build():
    import concourse.mybir as mybir
    import concourse.tile as tile
    from concourse import bacc

    nc = bacc.Bacc(None, target_bir_lowering=False)
    # xT4[p, bq, ko, fd] = x[bq*FD + fd, ko*P + p]
    xT = nc.declare_dram_parameter(
        "xT", [P, NB, KO, FD], mybir.dt.bfloat16, isOutput=False
    )
    # w3[p, ko, mo, c] = W[mo*P + c, ko*P + p]  (ko-major: bq0 consumes
    # weights one ko chunk at a time)
    w3 = nc.declare_dram_parameter(
        "w3", [P, KO, MO, P], mybir.dt.bfloat16, isOutput=False
    )
    bias = nc.declare_dram_parameter("bias", [P, MO], mybir.dt.float32, isOutput=False)
    # out4[p, bq, mo, fd] = y[bq*FD + fd, mo*P + p]
    out = nc.declare_dram_parameter(
        "out", [P, NB, MO, FD], mybir.dt.float32, isOutput=True
    )

    with tile.TileContext(nc) as tc:
        with (
            tc.tile_pool(name="const", bufs=1) as cpool,
            tc.tile_pool(name="outp", bufs=3) as opool,
            tc.tile_pool(name="psum", bufs=8, space="PSUM") as ppool,
        ):
            x_sb = cpool.tile([P, NB, KO, FD], mybir.dt.bfloat16)
            w_sb = cpool.tile([P, KO, MO, P], mybir.dt.bfloat16)
            b_sb = cpool.tile([P, MO], mybir.dt.float32)
            # PE HAM warm-up (shares the psum pool slots with the real
            # accumulation tiles; it finishes before they are needed).
            wu_sb = cpool.tile([P, 256], mybir.dt.bfloat16)
            nc.any.memset(wu_sb[:], 0.0)
            wu_ps = ppool.tile([P, FD], mybir.dt.float32, tag="ps")
            for _ in range(16):
                nc.tensor.matmul(
                    wu_ps[:, :256], wu_sb[:, :P], wu_sb[:], start=True, stop=True
                )
            # DMA issue order matches consumption order: (w, x) chunk pairs
            # for bq0 one ko at a time, then the remaining batch chunks as
            # one large contiguous DMA each.
            for ko in range(KO):
                nc.sync.dma_start(w_sb[:, ko], w3[:, ko])
                nc.sync.dma_start(x_sb[:, 0, ko], xT[:, 0, ko])
            nc.sync.dma_start(b_sb[:], bias[:])
            for bq in range(1, NB):
                nc.sync.dma_start(x_sb[:, bq], xT[:, bq])

            # bq0: contraction-outer over all 8 PSUM banks.
            ps0 = [
                ppool.tile([P, FD], mybir.dt.float32, tag="ps", name=f"ps0_{mo}")
                for mo in range(MO)
            ]
            o_sb = opool.tile([P, MO, FD], mybir.dt.float32)
            for ko in range(KO):
                for mo in range(MO):
                    nc.tensor.matmul(
                        ps0[mo][:],
                        w_sb[:, ko, mo],
                        x_sb[:, 0, ko],
                        start=(ko == 0),
                        stop=(ko == KO - 1),
                    )
            for mo in range(MO):
                nc.scalar.activation(
                    o_sb[:, mo],
                    ps0[mo][:],
                    mybir.ActivationFunctionType.Identity,
                    bias=b_sb[:, mo : mo + 1],
                )
            nc.sync.dma_start(out[:, 0], o_sb[:])

            # bq1..3: output-tile-outer, one PSUM bank at a time.
            for bq in range(1, NB):
                o_sb = opool.tile([P, MO, FD], mybir.dt.float32)
                for mo in range(MO):
                    ps = ppool.tile([P, FD], mybir.dt.float32, tag="ps")
                    for ko in range(KO):
                        nc.tensor.matmul(
                            ps[:],
                            w_sb[:, ko, mo],
                            x_sb[:, bq, ko],
                            start=(ko == 0),
                            stop=(ko == KO - 1),
                        )
                    nc.scalar.activation(
                        o_sb[:, mo],
                        ps[:],
                        mybir.ActivationFunctionType.Identity,
                        bias=b_sb[:, mo : mo + 1],
                    )
                if bq < NB - 1:
                    nc.sync.dma_start(out[:, bq], o_sb[:])
                else:
                    # Finer pushes on the last chunk so the final store
                    # doesn't add a 1 MiB DMA to the kernel tail.
                    for mh in range(0, MO, 2):
                        nc.sync.dma_start(
                            out[:, bq, mh : mh + 2], o_sb[:, mh : mh + 2]
                        )

    nc.compile()
    return nc


def kernel(x, weight, bias):
    global LAST_RESULT
    from concourse.bass_utils import run_bass_kernel_spmd

    if "nc" not in _cache:
        _cache["nc"] = _build()
    nc = _cache["nc"]

    bf16 = ml_dtypes.bfloat16
    # w3[p, ko, mo, c] = W[mo*P + c, ko*P + p]
    wb = weight.astype(bf16).reshape(MO, P, KO, P)  # [mo, c, ko, p]
    w3 = np.ascontiguousarray(wb.transpose(3, 2, 0, 1))  # [p, ko, mo, c]
    # bias laid out [P, MO]: b[p, mo] = bias[mo*P + p]
    b_t = np.ascontiguousarray(bias.astype(np.float32).reshape(MO, P).T)

    in_maps = []
    for c in range(NCORES):
        xs = x[c * BS : (c + 1) * BS].astype(bf16)
        # xT4[p, bq, ko, fd] = x[bq*FD + fd, ko*P + p]
        xr = xs.reshape(NB, FD, KO, P)  # [bq, fd, ko, p]
        xT = np.ascontiguousarray(xr.transpose(3, 0, 2, 1))  # [p, bq, ko, fd]
        in_maps.append({"xT": xT, "w3": w3, "bias": b_t})

    res = run_bass_kernel_spmd(nc, in_maps, list(range(NCORES)))
    LAST_RESULT = res

    y = np.empty((BATCH, OUT_F), dtype=np.float32)
    for c in range(NCORES):
        o = res.results[c]["out"]  # [p, bq, mo, fd]
        y[c * BS : (c + 1) * BS] = o.transpose(1, 3, 2, 0).reshape(BS, OUT_F)
    return y
